# revision 4
# baseline (speedup 1.0000x reference)
"""Cross-attention Trainium2 kernel v2 (8 NeuronCores, SPMD over Q rows).

Device computes ONLY the two big attention matmuls (scores + unnormalized
PV aggregation); everything that is O(N*E*E) or smaller runs on host:

    host:   G  = scale * Wq^T Wk               [E, E]
            C  = G^T @ m1^T                    [E, N1]  (per-core slice [E, QC])
            d2 = scale * m2 @ (Wk^T bq)        [N2]     (only if bq != 0)
    device: ST  = m2 @ C_c                     [N2, QC] scores (transposed layout)
            EST = exp(ST (+ d2 per-k bias))
            esum[p, q] = sum over k-tiles of EST    (gpsimd partial rowsums)
            UT  = m2^T @ EST                   [E, QC]
    host:   s   = esum.sum(0)                  softmax denominators
            out = (UT / s)^T @ Wv^T + bv       [QC, H]

The bk term of the scores is constant per query row -> softmax-invariant,
dropped exactly. PE work per core: 2*ktb*et*qch*nb matmuls (N=qw each) and
nothing else; row sums ride the idle gpsimd/Pool engine, normalization and
projections ride the host.
"""

import contextlib
import numpy as np

E = 1024
H = 1024
N1 = 8192
N2 = 8192
NCORES = 8
QC = N1 // NCORES
SCALE = 1.0 / np.sqrt(np.float32(H))

_BUILD_CACHE = {}


def _build(mm="f32r", biased_q=False, e=E, qc=QC, n2=N2, kb=512, repeat=1,
           repeat_py=False, st_bufs=3, ut_bufs=3, ug=2, c_bufs=1,
           est_bufs=3, m2_bufs=3, m2t_bufs=2, qw=None):
    """Build (and finalize) the per-core Bass kernel. Returns nc."""
    import concourse.bacc as bacc
    import concourse.tile as tile
    import concourse.mybir as mybir

    f32 = mybir.dt.float32
    rdt = {"f32": f32, "f32r": mybir.dt.float32r, "f16": mybir.dt.float16,
           "bf16": mybir.dt.bfloat16}[mm]

    et = e // 128              # e tiles
    ktb = kb // 128            # k tiles per block
    nb = n2 // kb              # k blocks
    if qw is None:
        qw = 512
    qw = min(qw, qc)
    qch = qc // qw
    # PSUM is 8 banks of 2KB/partition; clamp pool depths to fit
    banks_per_tile = max(1, (qw * 4) // 2048)
    while (st_bufs + ut_bufs) * banks_per_tile > 8:
        if st_bufs >= ut_bufs and st_bufs > 2:
            st_bufs -= 1
        elif ut_bufs > 2:
            ut_bufs -= 1
        else:
            break
    ng = nb // ug              # UT accumulation groups
    EXP = mybir.ActivationFunctionType.Exp

    nc = bacc.Bacc(None, target_bir_lowering=False)

    c_d = nc.dram_tensor("c", [e, qc], rdt, kind="ExternalInput")
    m2_d = nc.dram_tensor("m2", [n2, e], rdt, kind="ExternalInput")
    m2t_d = nc.dram_tensor("m2t", [e, n2], rdt, kind="ExternalInput")
    d2_d = (nc.dram_tensor("d2", [128, nb * ktb], f32, kind="ExternalInput")
            if biased_q else None)
    ut_d = nc.dram_tensor("ut", [e, qc], f32, kind="ExternalOutput")
    esum_d = nc.dram_tensor("esum", [128, qc], f32, kind="ExternalOutput")

    with tile.TileContext(nc) as tc:
        if repeat_py:
            reps = range(repeat)
        else:
            reps = [None]
        for _rep in reps:
            rep_ctx = (tc.For_i(0, repeat, 1)
                       if (repeat > 1 and not repeat_py)
                       else contextlib.nullcontext())
            with rep_ctx, tc.tile_pool(name="res", bufs=1) as res, \
                    tc.tile_pool(name="cp", bufs=c_bufs) as cpool:
                ut_sb = res.tile([128, et, qc], f32)
                esum_sb = res.tile([128, qc], f32)
                c_sb = cpool.tile([128, et, qc], rdt, tag="c")
                for t in range(et):
                    eng = nc.scalar if (t % 2) else nc.sync
                    eng.dma_start(out=c_sb[:, t, :],
                                  in_=c_d[t * 128:(t + 1) * 128, :])
                if biased_q:
                    d2_sb = res.tile([128, nb * ktb], f32)
                    nc.sync.dma_start(out=d2_sb, in_=d2_d[:, :])

                with (
                    tc.tile_pool(name="m2tp", bufs=m2t_bufs) as m2tp,
                    tc.tile_pool(name="m2p", bufs=m2_bufs) as m2p,
                    tc.tile_pool(name="estp", bufs=est_bufs) as estp,
                    tc.tile_pool(name="stps", bufs=st_bufs, space="PSUM") as stps,
                    tc.tile_pool(name="utps", bufs=ut_bufs, space="PSUM") as utps,
                ):
                    for g in range(ng):
                        est_list = []
                        m2_list = []
                        for bb in range(ug):
                            b = g * ug + bb
                            k0 = b * kb
                            m2t_blk = m2tp.tile([128, et, kb], rdt, tag="m2t")
                            for t in range(et):
                                eng = nc.scalar if (t % 2) else nc.sync
                                eng.dma_start(
                                    out=m2t_blk[:, t, :],
                                    in_=m2t_d[t * 128:(t + 1) * 128, k0:k0 + kb])
                            m2_blk = m2p.tile([128, ktb, e], rdt, tag="m2")
                            for j in range(ktb):
                                eng = nc.sync if (j % 2) else nc.scalar
                                eng.dma_start(
                                    out=m2_blk[:, j, :],
                                    in_=m2_d[k0 + j * 128:k0 + (j + 1) * 128, :])
                            est = estp.tile([128, ktb, qc], rdt, tag="est")

                            # scores^T block (+ per-k bias), exp
                            for j in range(ktb):
                                for q0 in range(qch):
                                    stp = stps.tile([128, qw], f32, tag="st")
                                    for t in range(et):
                                        nc.tensor.matmul(
                                            stp,
                                            m2t_blk[:, t, j * 128:(j + 1) * 128],
                                            c_sb[:, t, q0 * qw:(q0 + 1) * qw],
                                            start=(t == 0), stop=(t == et - 1),
                                        )
                                    nc.scalar.activation(
                                        est[:, j, q0 * qw:(q0 + 1) * qw], stp,
                                        EXP,
                                        bias=(d2_sb[:, b * ktb + j:b * ktb + j + 1]
                                              if biased_q else 0.0),
                                        scale=1.0)
                                # softmax denominator partials on gpsimd
                                ej = est[:, j, :]
                                if mm == "f32r":
                                    ej = ej.bitcast(f32)
                                if b == 0 and j == 0:
                                    nc.gpsimd.tensor_copy(esum_sb, ej)
                                else:
                                    nc.gpsimd.tensor_add(esum_sb, esum_sb, ej)
                            est_list.append(est)
                            m2_list.append(m2_blk)

                        # UT += m2^T @ EST over the whole group in PSUM
                        for t in range(et):
                            for q0 in range(qch):
                                up = utps.tile([128, qw], f32, tag="ut")
                                for bb in range(ug):
                                    for j in range(ktb):
                                        nc.tensor.matmul(
                                            up,
                                            m2_list[bb][:, j, t * 128:(t + 1) * 128],
                                            est_list[bb][:, j, q0 * qw:(q0 + 1) * qw],
                                            start=(bb == 0 and j == 0),
                                            stop=(bb == ug - 1 and j == ktb - 1),
                                        )
                                dst = ut_sb[:, t, q0 * qw:(q0 + 1) * qw]
                                if g == 0:
                                    nc.vector.tensor_copy(dst, up)
                                else:
                                    nc.vector.tensor_add(dst, dst, up)

                nc.gpsimd.dma_start(out=esum_d[:, :], in_=esum_sb)
                for t in range(et):
                    nc.gpsimd.dma_start(out=ut_d[t * 128:(t + 1) * 128, :],
                                        in_=ut_sb[:, t, :])

    nc.finalize()
    return nc


def _get_nc(key):
    if key not in _BUILD_CACHE:
        _BUILD_CACHE[key] = _build(**dict(key))
    return _BUILD_CACHE[key]


def _prep_inputs(molecule1, molecule2, Wq, bq, Wk, bk, Wv, bv, mm="f32r"):
    """Host-side prep. Returns (in_maps, biased_q, host_ctx)."""
    m1 = np.asarray(molecule1, np.float32)
    m2 = np.ascontiguousarray(np.asarray(molecule2, np.float32))
    wq = np.asarray(Wq, np.float64)
    wk = np.asarray(Wk, np.float64)
    wv = np.asarray(Wv, np.float32)
    bq64 = np.asarray(bq, np.float64)
    bv32 = np.asarray(bv, np.float32)

    scale = 1.0 / np.sqrt(np.float64(wq.shape[0]))
    g = (scale * (wq.T @ wk)).astype(np.float32)       # [E, E]
    c_all = np.ascontiguousarray((m1 @ g).T)           # [E, N1]
    m2t = np.ascontiguousarray(m2.T)

    v2 = (scale * (wk.T @ bq64)).astype(np.float32)
    d2 = m2 @ v2                                       # [N2]
    biased_q = bool(np.any(d2))

    cast = (lambda a: a.astype(np.float16)) if mm == "f16" else (lambda a: a)
    m2c = cast(m2)
    m2tc = cast(m2t)

    qc = m1.shape[0] // NCORES
    in_maps = []
    for c in range(NCORES):
        m = {
            "c": cast(np.ascontiguousarray(c_all[:, c * qc:(c + 1) * qc])),
            "m2": m2c,
            "m2t": m2tc,
        }
        if biased_q:
            # per-k exp bias, laid out [128, ktb*nb] matching k-tile order
            m["d2"] = np.ascontiguousarray(
                d2.reshape(-1, 128).T).astype(np.float32)
        in_maps.append(m)
    host_ctx = (wv, bv32)
    return in_maps, biased_q, host_ctx


def _finish(results, host_ctx, out_dtype):
    """Host epilogue: normalize and apply the V projection."""
    wv, bv32 = host_ctx
    outs = []
    for c in range(NCORES):
        ut = results[c]["ut"]            # [E, QC] f32
        esum = results[c]["esum"]        # [128, QC] f32
        s = esum.sum(axis=0)             # [QC]
        utn = ut / s[None, :]
        outs.append(utn.T @ wv.T + bv32)
    out = np.concatenate(outs, axis=0)
    return out.astype(out_dtype, copy=False)


def kernel(molecule1, molecule2, Wq, bq, Wk, bk, Wv, bv):
    from concourse.bass_utils import run_bass_kernel_spmd

    import os
    mm = os.environ.get("BASS_MM", "f32r")
    in_maps, biased_q, host_ctx = _prep_inputs(
        molecule1, molecule2, Wq, bq, Wk, bk, Wv, bv, mm=mm)
    kb = 1024 if mm == "f16" else 512
    key = (("mm", mm), ("biased_q", biased_q), ("kb", kb))
    nc = _get_nc(key)
    res = run_bass_kernel_spmd(nc, in_maps, core_ids=list(range(NCORES)))
    return _finish(res.results, host_ctx, np.asarray(molecule1).dtype)
